# revision 1
# baseline (speedup 1.0000x reference)
"""Fused ParallelTransformerBlock kernel for 8 Trainium2 NeuronCores.

Sharding: Megatron-style tensor-parallel (2-way over heads + mlp_hidden)
x data-parallel (4-way over batch). Core c handles batch c//2 with
head/mlp shard c%2. Each core computes a partial output of linear2; the
residual x is folded in on the tp==0 core (tp==1 cores get zeros), and
the host sums the two partials per batch.

All matmuls run in float32r (full PE rate, ~1.5e-4 rel err); everything
else is fp32.
"""
import numpy as np

import concourse.bass as bass
import concourse.tile as tile
from concourse import bacc, mybir

DIM = 1024
L = 2048
B = 4
H = 16
DH = 64
MLP = 3072
EPS_LN = 1e-6
EPS_RMS = 1e-6

P = 128
KD = DIM // P          # 8 k-tiles over model dim
TT = L // P            # 16 token tiles
HL = H // 2            # 8 heads per core
NP = HL // 2           # 4 head pairs
MLPL = MLP // 2        # 1536 mlp columns per core
FT = MLPL // P         # 12 mlp feature tiles
AKT = HL * DH // P     # 4 attn k-tiles into linear2
KT2 = AKT + FT         # 16 linear2 k-tiles

F32 = mybir.dt.float32
F32R = mybir.dt.float32r
AF = mybir.ActivationFunctionType
ALU = mybir.AluOpType
AX = mybir.AxisListType


def gen_program(repeat: int = 1):
    nc = bacc.Bacc("TRN2", target_bir_lowering=False, debug=False, num_devices=8)

    x = nc.dram_tensor("x", (L, DIM), F32, kind="ExternalInput")
    xres = nc.dram_tensor("xres", (L, DIM), F32, kind="ExternalInput")
    peq = nc.dram_tensor("peq", (L, 2, DH), F32, kind="ExternalInput")
    pek = nc.dram_tensor("pek", (L, 2, DH), F32, kind="ExternalInput")
    w1qkv = nc.dram_tensor("w1qkv", (DIM, 3 * HL * DH), F32R, kind="ExternalInput")
    w1mlp = nc.dram_tensor("w1mlp", (DIM, MLPL), F32R, kind="ExternalInput")
    w2 = nc.dram_tensor("w2", (HL * DH + MLPL, DIM), F32R, kind="ExternalInput")
    ident_in = nc.dram_tensor("ident", (P, P), F32R, kind="ExternalInput")
    ones_in = nc.dram_tensor("ones_d", (1, L), F32R, kind="ExternalInput")
    y = nc.dram_tensor("y", (L, DIM), F32, kind="ExternalOutput")

    from contextlib import ExitStack
    with tile.TileContext(nc) as tc, ExitStack() as es:
        pool_const = es.enter_context(tc.tile_pool(name="const", bufs=1))
        pool_w1024 = es.enter_context(tc.tile_pool(name="w1024", bufs=3))
        pool_w512 = es.enter_context(tc.tile_pool(name="w512", bufs=4))
        pool_small = es.enter_context(tc.tile_pool(name="small", bufs=8))
        pool_ps1024 = es.enter_context(tc.tile_pool(name="ps1024", bufs=2, space="PSUM"))
        pool_ps512 = es.enter_context(tc.tile_pool(name="ps512", bufs=4, space="PSUM"))
        pool_dram = es.enter_context(tc.tile_pool(name="dram", bufs=1, space="DRAM"))

        ident = pool_const.tile([P, P], F32R, tag="ident")
        nc.sync.dma_start(ident, ident_in[:, :])
        epsc = pool_const.tile([P, 1], F32, tag="epsc")
        nc.vector.memset(epsc, EPS_LN)
        sume = pool_const.tile([HL, L], F32, tag="sume")

        def body():
            v_d = pool_dram.tile([HL, L, DH + 1], F32R, tag="v_d")
            nc.sync.dma_start(
                v_d[:, :, DH:DH + 1],
                ones_in[0, :].unsqueeze(0).broadcast_to([HL, L]).unsqueeze(-1))
            mlp_d = pool_dram.tile([FT, P, L], F32R, tag="mlp_d")
            sume_d = pool_dram.tile([HL, L], F32, tag="sume_d")
            rec_d = pool_dram.tile([HL, L], F32, tag="rec_d")
            es_xT = ExitStack()
            pool_xT = es_xT.enter_context(tc.tile_pool(name="pxT", bufs=1, side="right"))
            xT = pool_xT.tile([P, KD, L], F32R, tag="xT")

            # ---- Phase A: LayerNorm + transpose to [dim, tok] ----
            for tt in range(TT):
                ts = slice(tt * P, (tt + 1) * P)
                xt = pool_w1024.tile([P, DIM], F32, tag="w1024f")
                nc.sync.dma_start(xt, x[ts, :])
                st = pool_small.tile([P, 2, 6], F32, tag="st")
                nc.vector.bn_stats(st[:, 0, :], xt[:, 0:512])
                nc.vector.bn_stats(st[:, 1, :], xt[:, 512:1024])
                mv = pool_small.tile([P, 2], F32, tag="mv")
                nc.vector.bn_aggr(mv, st)
                std = pool_small.tile([P, 1], F32, tag="std")
                nc.scalar.activation(std, mv[:, 1:2], AF.Sqrt, bias=epsc)
                rstd = pool_small.tile([P, 1], F32, tag="rstd")
                nc.vector.reciprocal(rstd, std)
                xln = pool_w1024.tile([P, DIM], F32R, tag="w1024r")
                nc.vector.tensor_scalar(
                    out=xln, in0=xt, scalar1=mv[:, 0:1], scalar2=rstd,
                    op0=ALU.subtract, op1=ALU.mult,
                )
                for g in range(2):
                    pst = pool_ps512.tile([P, 512], F32R, tag="ps512")
                    for j in range(4):
                        kd = g * 4 + j
                        nc.tensor.transpose(
                            pst[:, j * P:(j + 1) * P],
                            xln[:, kd * P:(kd + 1) * P], ident)
                    nc.vector.tensor_copy(
                        xT[:, g * 4:(g + 1) * 4, ts],
                        pst.rearrange("p (j t) -> p j t", j=4))

            # ---- Phase B2: linear1 qkv + rmsnorm + rope + transpose ----
            es_qkT = ExitStack()
            pool_qkT = es_qkT.enter_context(tc.tile_pool(name="pqkT", bufs=1))
            qT = pool_qkT.tile([P, NP, L], F32R, tag="qT")
            kT = pool_qkT.tile([P, NP, L], F32R, tag="kT")
            es_b2 = ExitStack()
            pool_pe = es_b2.enter_context(tc.tile_pool(name="ppe", bufs=4))
            pool_v = es_b2.enter_context(tc.tile_pool(name="pv", bufs=2))
            pool_w1s = es_b2.enter_context(tc.tile_pool(name="pw1s", bufs=1))

            for part in range(3):  # 0=q, 1=k, 2=v
                w1t = pool_w1s.tile([P, KD, 512], F32R, tag="w1qkv")
                nc.sync.dma_start(
                    w1t, w1qkv[:, part * 512:(part + 1) * 512]
                    .rearrange("(kt p) f -> p kt f", p=P))
                for tt in range(TT):
                    ts = slice(tt * P, (tt + 1) * P)
                    ps = pool_ps512.tile([P, 512], F32, tag="ps512")
                    for kd in range(KD):
                        nc.tensor.matmul(ps, xT[:, kd, ts], w1t[:, kd],
                                         start=(kd == 0), stop=(kd == KD - 1))
                    if part == 2:  # v (ones column of v_d comes from ones_in)
                        vt = pool_v.tile([P, HL, DH], F32R, tag="vt")
                        nc.scalar.copy(
                            vt, ps.rearrange("p (h d) -> p h d", d=DH))
                        nc.sync.dma_start(
                            v_d[:, ts, 0:DH].rearrange("h t c -> t h c"), vt)
                        continue
                    # rmsnorm stats
                    sq = pool_w512.tile([P, 512], F32, tag="w512f")
                    nc.scalar.activation(sq, ps, AF.Square)
                    ss = pool_small.tile([P, HL], F32, tag="ss")
                    nc.vector.tensor_reduce(
                        ss, sq.rearrange("p (h d) -> p h d", d=DH),
                        axis=AX.X, op=ALU.add)
                    sd = pool_small.tile([P, HL], F32, tag="sd")
                    nc.scalar.activation(sd, ss, AF.Sqrt, scale=1.0 / DH,
                                         bias=epsc)
                    rs = pool_small.tile([P, HL], F32, tag="rs")
                    nc.vector.reciprocal(rs, sd)
                    # rope (pe pre-folded with q/k head scales on host)
                    pe_src = peq if part == 0 else pek
                    pet = pool_pe.tile([P, 2, DH], F32, tag="pe")
                    nc.sync.dma_start(pet, pe_src[ts, :, :])
                    qs = pool_w512.tile([P, 512], F32, tag="w512f")
                    nc.vector.tensor_copy(qs, ps)
                    qs4 = qs.rearrange("p (h d t) -> p h d t", d=DH // 2, t=2)
                    qe = qs4[:, :, :, 0].unsqueeze(-1).broadcast_to([P, HL, DH // 2, 2])
                    qo = qs4[:, :, :, 1].unsqueeze(-1).broadcast_to([P, HL, DH // 2, 2])
                    p0 = (pet[:, 0, :].rearrange("p (d t) -> p d t", t=2)
                          .unsqueeze(1).broadcast_to([P, HL, DH // 2, 2]))
                    p1 = (pet[:, 1, :].rearrange("p (d t) -> p d t", t=2)
                          .unsqueeze(1).broadcast_to([P, HL, DH // 2, 2]))
                    t1 = pool_w512.tile([P, 512], F32, tag="w512f")
                    t2 = pool_w512.tile([P, 512], F32, tag="w512f")
                    t1v = t1.rearrange("p (h d t) -> p h d t", d=DH // 2, t=2)
                    t2v = t2.rearrange("p (h d t) -> p h d t", d=DH // 2, t=2)
                    nc.gpsimd.tensor_tensor(out=t1v, in0=qe, in1=p0, op=ALU.mult)
                    nc.gpsimd.tensor_tensor(out=t2v, in0=qo, in1=p1, op=ALU.mult)
                    nc.gpsimd.tensor_tensor(out=t1, in0=t1, in1=t2, op=ALU.add)
                    rq = pool_w512.tile([P, 512], F32R, tag="w512r")
                    rqv = rq.rearrange("p (h d) -> p h d", d=DH)
                    rsb = rs.unsqueeze(-1).broadcast_to([P, HL, DH])
                    nc.vector.tensor_tensor(
                        out=rqv, in0=t1.rearrange("p (h d) -> p h d", d=DH),
                        in1=rsb, op=ALU.mult)
                    # transpose head pairs -> qT/kT [128, pair, tok]
                    dst = qT if part == 0 else kT
                    pst = pool_ps512.tile([P, 512], F32R, tag="ps512")
                    for pr in range(NP):
                        nc.tensor.transpose(
                            pst[:, pr * P:(pr + 1) * P],
                            rq[:, pr * P:(pr + 1) * P], ident)
                    nc.vector.tensor_copy(
                        dst[:, :, ts], pst.rearrange("p (j t) -> p j t", j=NP))
            es_b2.close()

            # ---- Phase B1: linear1 mlp + gelu -> DRAM ----
            es_b1 = ExitStack()
            pool_w1m = es_b1.enter_context(tc.tile_pool(name="pw1m", bufs=3))
            for ft in range(FT):
                w1t = pool_w1m.tile([P, KD, P], F32R, tag="w1mlp")
                nc.sync.dma_start(
                    w1t, w1mlp[:, ft * P:(ft + 1) * P]
                    .rearrange("(kt p) f -> p kt f", p=P))
                for tm in range(2):
                    ps = pool_ps1024.tile([P, 1024], F32, tag="ps1024")
                    for tn in range(2):
                        off = tm * 1024 + tn * 512
                        for kd in range(KD):
                            nc.tensor.matmul(
                                ps[:, tn * 512:(tn + 1) * 512],
                                w1t[:, kd], xT[:, kd, off:off + 512],
                                start=(kd == 0), stop=(kd == KD - 1))
                    gt = pool_w1024.tile([P, 1024], F32R, tag="w1024r")
                    nc.scalar.activation(gt, ps, AF.Gelu_apprx_tanh)
                    nc.sync.dma_start(
                        mlp_d[ft, :, tm * 1024:(tm + 1) * 1024], gt)
            es_b1.close()
            es_xT.close()

            # ---- Phase C: attention ----
            es_attn = ExitStack()
            pool_attn = es_attn.enter_context(tc.tile_pool(name="pattn", bufs=1, side="right"))
            attnT = pool_attn.tile([P, NP, L], F32R, tag="attnT")
            es_c = ExitStack()
            pool_vr = es_c.enter_context(tc.tile_pool(name="pvr", bufs=2, side="right"))
            for h in range(HL):
                ph, sub = h // 2, h % 2
                bp = sub * DH
                set_t = pool_vr.tile([P, 4, 512], F32, tag="sexp")
                vh = pool_vr.tile([P, TT, DH + 1], F32R, tag="vr")
                nc.sync.dma_start(
                    vh, v_d[h].rearrange("(kt p) c -> p kt c", p=P))
                for qc2 in range(2):
                    pa = []
                    for qn in range(2):
                        pa.append(pool_ps512.tile([P, 512], F32, tag="ps512", name=f"pa{qn}"))
                    for kt in range(TT):
                        pss = pool_ps1024.tile([P, 1024], F32, tag="ps1024")
                        for qn in range(2):
                            qoff = qc2 * 1024 + qn * 512
                            nc.tensor.matmul(
                                pss[:, qn * 512:(qn + 1) * 512],
                                kT[bp:bp + DH, ph, kt * P:(kt + 1) * P],
                                qT[bp:bp + DH, ph, qoff:qoff + 512],
                                start=True, stop=True)
                        ex = pool_w1024.tile([P, 1024], F32R, tag="w1024r")
                        nc.scalar.activation(ex, pss, AF.Exp, scale=0.125)
                        for qn in range(2):
                            nc.tensor.matmul(
                                pa[qn][0:DH + 1, :], vh[:, kt, :],
                                ex[:, qn * 512:(qn + 1) * 512],
                                start=(kt == 0), stop=(kt == TT - 1))
                    for qn in range(2):
                        qoff = qc2 * 1024 + qn * 512
                        qc = qc2 * 2 + qn
                        nc.scalar.copy(
                            attnT[bp:bp + DH, ph, qoff:qoff + 512],
                            pa[qn][0:DH, :])
                        nc.vector.tensor_copy(
                            set_t[64:65, qc, :], pa[qn][DH:DH + 1, :])
                nc.sync.dma_start(
                    sume_d[h:h + 1, :],
                    set_t[64:65].rearrange("p c t -> p (c t)"))
            nc.sync.dma_start(sume, sume_d[:, :])
            nc.vector.reciprocal(sume, sume)
            nc.sync.dma_start(rec_d[:, :], sume)
            for h in range(HL):
                ph, sub = h // 2, h % 2
                bp = sub * DH
                for qc in range(4):
                    qs_ = slice(qc * 512, (qc + 1) * 512)
                    bc = pool_w512.tile([P, 512], F32, tag="w512f")
                    nc.sync.dma_start(
                        bc, rec_d[h, qs_].partition_broadcast(P))
                    nc.vector.tensor_tensor(
                        out=attnT[bp:bp + DH, ph, qs_],
                        in0=attnT[bp:bp + DH, ph, qs_],
                        in1=bc[bp:bp + DH, :], op=ALU.mult)
            es_c.close()
            es_qkT.close()

            # ---- Phase E: linear2 + residual ----
            es_e = ExitStack()
            pool_w2 = es_e.enter_context(tc.tile_pool(name="pw2", bufs=1, side="right"))
            pool_mlp = es_e.enter_context(tc.tile_pool(name="pmlp", bufs=2, side="right"))
            w2sb = pool_w2.tile([P, KT2, DIM], F32R, tag="w2")
            nc.sync.dma_start(w2sb, w2.rearrange("(kt p) o -> p kt o", p=P))
            for tc_ in range(2):
                msb = []
                for half in range(2):
                    m = pool_mlp.tile([P, FT // 2, 1024], F32R, tag="mlp")
                    nc.sync.dma_start(
                        m, mlp_d[half * (FT // 2):(half + 1) * (FT // 2), :,
                                 tc_ * 1024:(tc_ + 1) * 1024]
                        .rearrange("f p t -> p f t"))
                    msb.append(m)
                for t8 in range(8):
                    tt = tc_ * 8 + t8
                    ts = slice(tt * P, (tt + 1) * P)
                    xr = pool_w1024.tile([P, DIM], F32, tag="w1024f")
                    nc.sync.dma_start(xr, xres[ts, :])
                    yt = pool_w1024.tile([P, DIM], F32, tag="w1024f")
                    for oc in range(2):
                        ps = pool_ps512.tile([P, 512], F32, tag="ps512")
                        for pr in range(AKT):
                            nc.tensor.matmul(
                                ps, attnT[:, pr, ts],
                                w2sb[:, pr, oc * 512:(oc + 1) * 512],
                                start=(pr == 0), stop=False)
                        for ft in range(FT):
                            m = msb[ft // (FT // 2)]
                            nc.tensor.matmul(
                                ps, m[:, ft % (FT // 2), t8 * P:(t8 + 1) * P],
                                w2sb[:, AKT + ft, oc * 512:(oc + 1) * 512],
                                start=False, stop=(ft == FT - 1))
                        nc.vector.tensor_tensor(
                            out=yt[:, oc * 512:(oc + 1) * 512], in0=ps,
                            in1=xr[:, oc * 512:(oc + 1) * 512], op=ALU.add)
                    nc.sync.dma_start(y[ts, :], yt)
            es_e.close()
            es_attn.close()

        if repeat == 1:
            body()
        else:
            with tc.For_i(0, repeat, 1):
                body()

    nc.finalize()
    return nc


# ---------------- host side ----------------

_NC_CACHE = {}


def _get_nc(repeat=1):
    if repeat not in _NC_CACHE:
        _NC_CACHE[repeat] = gen_program(repeat)
    return _NC_CACHE[repeat]


def make_in_maps(x, pe, W1, b1, W2, b2, q_scale, k_scale):
    x = np.asarray(x, dtype=np.float32)
    pe = np.asarray(pe, dtype=np.float32)
    W1 = np.asarray(W1, dtype=np.float32)
    W2 = np.asarray(W2, dtype=np.float32)
    q_scale = np.asarray(q_scale, dtype=np.float32)
    k_scale = np.asarray(k_scale, dtype=np.float32)
    assert not np.any(np.asarray(b1)), "kernel assumes b1 == 0"

    pe_r = pe.reshape(L, DH // 2, 2, 2)

    def fold_pe(scale):
        s0 = np.repeat(scale[0::2], 2)  # scale for even input element
        s1 = np.repeat(scale[1::2], 2)
        p0 = pe_r[..., 0].reshape(L, DH) * s0[None, :]
        p1 = pe_r[..., 1].reshape(L, DH) * s1[None, :]
        return np.ascontiguousarray(
            np.stack([p0, p1], axis=1).astype(np.float32))

    peq = fold_pe(q_scale)
    pek = fold_pe(k_scale)
    zeros = np.zeros((L, DIM), dtype=np.float32)

    in_maps = []
    for c in range(8):
        b_idx, tp = c // 2, c % 2
        hs = tp * 512
        w1qkv = np.ascontiguousarray(np.concatenate(
            [W1[:, hs:hs + 512],
             W1[:, DIM + hs:DIM + hs + 512],
             W1[:, 2 * DIM + hs:2 * DIM + hs + 512]], axis=1))
        w1mlp = np.ascontiguousarray(W1[:, 3 * DIM + tp * MLPL:3 * DIM + (tp + 1) * MLPL])
        w2sh = np.ascontiguousarray(np.concatenate(
            [W2[hs:hs + 512, :],
             W2[DIM + tp * MLPL:DIM + (tp + 1) * MLPL, :]], axis=0))
        in_maps.append({
            "x": np.ascontiguousarray(x[b_idx]),
            "xres": np.ascontiguousarray(x[b_idx]) if tp == 0 else zeros,
            "peq": peq, "pek": pek,
            "w1qkv": w1qkv, "w1mlp": w1mlp, "w2": w2sh,
            "ident": np.eye(P, dtype=np.float32),
            "ones_d": np.ones((1, L), dtype=np.float32),
        })
    return in_maps


def combine_outputs(results, b2):
    b2 = np.asarray(b2, dtype=np.float32)
    y = np.empty((B, L, DIM), dtype=np.float32)
    for b_idx in range(B):
        y[b_idx] = results[2 * b_idx]["y"] + results[2 * b_idx + 1]["y"] + b2[None, :]
    return y


def kernel(x, pe, W1, b1, W2, b2, q_scale, k_scale):
    from concourse.bass_utils import run_bass_kernel_spmd
    nc = _get_nc(repeat=1)
    in_maps = make_in_maps(x, pe, W1, b1, W2, b2, q_scale, k_scale)
    res = run_bass_kernel_spmd(nc, in_maps, core_ids=list(range(8)))
    return combine_outputs(res.results, b2)



# revision 15
# speedup vs baseline: 4.2882x; 4.2882x over previous
"""Fused ParallelTransformerBlock kernel for 8 Trainium2 NeuronCores.

Sharding: Megatron-style tensor-parallel (2-way over heads + mlp_hidden)
x data-parallel (4-way over batch). Core c handles batch c//2 with
head/mlp shard c%2. Each core computes a partial output of linear2
(no residual, no bias); the host sums the two partials per batch and
adds x + b2 in fp32.

All tensors are bf16 except PSUM accumulation, layer/rms-norm stats and
the final output (fp32). All intermediates stay in SBUF (no DRAM
round-trips). QK^T matmuls for a head pair run row-tiled (base
partitions 0 and 64) so the two K=64 matmuls overlap on the PE array.
"""
import numpy as np

import concourse.bass as bass
import concourse.tile as tile
from concourse import bacc, mybir

DIM = 1024
L = 2048
B = 4
H = 16
DH = 64
MLP = 3072
EPS_LN = 1e-6
EPS_RMS = 1e-6

P = 128
KD = DIM // P          # 8 k-tiles over model dim
TT = L // P            # 16 token tiles
HL = H // 2            # 8 heads per core
NP = HL // 2           # 4 head pairs
MLPL = MLP // 2        # 1536 mlp columns per core
FT = MLPL // P         # 12 mlp feature tiles
AKT = HL * DH // P     # 4 attn k-tiles into linear2
KT2 = AKT + FT         # 16 linear2 k-tiles

F32 = mybir.dt.float32
BF16 = mybir.dt.bfloat16
AF = mybir.ActivationFunctionType
ALU = mybir.AluOpType
AX = mybir.AxisListType


def gen_program(repeat: int = 1):
    nc = bacc.Bacc("TRN2", target_bir_lowering=False, debug=False, num_devices=8)

    xb = nc.dram_tensor("xb", (L, DIM), BF16, kind="ExternalInput")
    peq = nc.dram_tensor("peq", (L, 2, DH), BF16, kind="ExternalInput")
    pek = nc.dram_tensor("pek", (L, 2, DH), BF16, kind="ExternalInput")
    w1qkv = nc.dram_tensor("w1qkv", (DIM, 3 * HL * DH), BF16, kind="ExternalInput")
    w1mlp = nc.dram_tensor("w1mlp", (DIM, MLPL), BF16, kind="ExternalInput")
    w2 = nc.dram_tensor("w2", (HL * DH + MLPL, DIM), BF16, kind="ExternalInput")
    ident_in = nc.dram_tensor("ident", (P, P), BF16, kind="ExternalInput")
    y = nc.dram_tensor("y", (L, DIM), F32, kind="ExternalOutput")

    from contextlib import ExitStack
    with tile.TileContext(nc) as tc, ExitStack() as es:
        pool_const = es.enter_context(tc.tile_pool(name="const", bufs=1))
        pool_w1024 = es.enter_context(tc.tile_pool(name="w1024", bufs=3))
        pool_w512 = es.enter_context(tc.tile_pool(name="w512", bufs=5))
        pool_small = es.enter_context(tc.tile_pool(name="small", bufs=8))
        pool_ps512 = es.enter_context(tc.tile_pool(name="ps512", bufs=4, space="PSUM"))
        pool_psb = es.enter_context(tc.tile_pool(name="psb", bufs=2, space="PSUM"))

        ident = pool_const.tile([P, P], BF16, tag="ident")
        nc.sync.dma_start(ident, ident_in[:, :])
        epsc = pool_const.tile([P, 1], F32, tag="epsc")
        nc.vector.memset(epsc, EPS_LN)
        ones_sb = pool_const.tile([P, DH], BF16, tag="ones_sb")
        nc.vector.memset(ones_sb, 1.0)

        def body():
            es_res = ExitStack()
            pool_res = es_res.enter_context(
                tc.tile_pool(name="pres", bufs=1, side="right"))
            # v laid out per (kt, head) as [tok-in-tile, kt, h, DH+1] with a
            # ones column at the end (accumulates the softmax denominator).
            v_sb = pool_res.tile([P, TT, HL, DH + 1], BF16, tag="v_sb")
            nc.vector.memset(v_sb[:, :, :, DH:DH + 1], 1.0)
            mlp_sb = pool_res.tile([P, FT, L], BF16, tag="mlp_sb")

            es_xT = ExitStack()
            pool_xT = es_xT.enter_context(tc.tile_pool(name="pxT", bufs=1))
            xT = pool_xT.tile([P, KD, L], BF16, tag="xT")
            w1m = pool_xT.tile([P, KD, MLPL], BF16, tag="w1m")
            nc.sync.dma_start(w1m, w1mlp.rearrange("(kt p) f -> p kt f", p=P))

            es_w1 = ExitStack()
            pool_wq = es_w1.enter_context(tc.tile_pool(name="pwq", bufs=1))
            w1sb = pool_wq.tile([P, KD, 3 * HL * DH], BF16, tag="w1sb")
            nc.sync.dma_start(w1sb, w1qkv.rearrange("(kt p) f -> p kt f", p=P))

            # ---- Phase A: LayerNorm + transpose to [dim, tok] ----
            for tt in range(TT):
                ts = slice(tt * P, (tt + 1) * P)
                xt = pool_w1024.tile([P, DIM], BF16, tag="w1024b")
                nc.sync.dma_start(xt, xb[ts, :])
                st = pool_small.tile([P, 2, 6], F32, tag="st")
                nc.vector.bn_stats(st[:, 0, :], xt[:, 0:512])
                nc.vector.bn_stats(st[:, 1, :], xt[:, 512:1024])
                mv = pool_small.tile([P, 2], F32, tag="mv")
                nc.vector.bn_aggr(mv, st)
                std = pool_small.tile([P, 1], F32, tag="std")
                nc.scalar.activation(std, mv[:, 1:2], AF.Sqrt, bias=epsc)
                rstd = pool_small.tile([P, 1], F32, tag="rstd")
                nc.vector.reciprocal(rstd, std)
                xln = pool_w1024.tile([P, DIM], BF16, tag="w1024b")
                nc.vector.tensor_scalar(
                    out=xln, in0=xt, scalar1=mv[:, 0:1], scalar2=rstd,
                    op0=ALU.subtract, op1=ALU.mult,
                )
                for g in range(2):
                    pst = pool_psb.tile([P, 512], BF16, tag="ps512b")
                    for j in range(4):
                        kd = g * 4 + j
                        nc.tensor.transpose(
                            pst[:, j * P:(j + 1) * P],
                            xln[:, kd * P:(kd + 1) * P], ident)
                    nc.vector.tensor_copy(
                        xT[:, g * 4:(g + 1) * 4, ts],
                        pst.rearrange("p (j t) -> p j t", j=4))

            # ---- Phase B2: linear1 qkv + rmsnorm + rope + transpose ----
            es_qkT = ExitStack()
            pool_qkT = es_qkT.enter_context(
                tc.tile_pool(name="pqkT", bufs=1, side="right"))
            qT = pool_qkT.tile([P, NP, L], BF16, tag="qT")
            kT = pool_qkT.tile([P, NP, L], BF16, tag="kT")

            for part in range(3):  # 0=q, 1=k, 2=v
                for tt in range(TT):
                    ts = slice(tt * P, (tt + 1) * P)
                    ps = pool_ps512.tile([P, 512], F32, tag="ps512")
                    for kd in range(KD):
                        nc.tensor.matmul(
                            ps, xT[:, kd, ts],
                            w1sb[:, kd, part * 512:(part + 1) * 512],
                            start=(kd == 0), stop=(kd == KD - 1))
                    if part == 2:  # v
                        nc.scalar.copy(
                            v_sb[:, tt, :, 0:DH],
                            ps.rearrange("p (h d) -> p h d", d=DH))
                        continue
                    # rmsnorm stats
                    sq = pool_w512.tile([P, 512], F32, tag="w512f")
                    nc.vector.tensor_tensor(out=sq, in0=ps, in1=ps, op=ALU.mult)
                    ss = pool_small.tile([P, HL], F32, tag="ss")
                    nc.vector.tensor_reduce(
                        ss, sq.rearrange("p (h d) -> p h d", d=DH),
                        axis=AX.X, op=ALU.add)
                    sd = pool_small.tile([P, HL], F32, tag="sd")
                    nc.scalar.activation(sd, ss, AF.Sqrt, scale=1.0 / DH,
                                         bias=epsc)
                    rs = pool_small.tile([P, HL], F32, tag="rs")
                    nc.vector.reciprocal(rs, sd)
                    # rope (pe pre-folded with q/k head scales on host)
                    pe_src = peq if part == 0 else pek
                    pet = pool_small.tile([P, 2, DH], BF16, tag="pet")
                    nc.sync.dma_start(pet, pe_src[ts, :, :])
                    qs = pool_w512.tile([P, 512], BF16, tag="w512b")
                    nc.vector.tensor_copy(qs, ps)
                    qs4 = qs.rearrange("p (h d t) -> p h d t", d=DH // 2, t=2)
                    qe = qs4[:, :, :, 0].unsqueeze(-1).broadcast_to([P, HL, DH // 2, 2])
                    qo = qs4[:, :, :, 1].unsqueeze(-1).broadcast_to([P, HL, DH // 2, 2])
                    p0 = (pet[:, 0, :].rearrange("p (d t) -> p d t", t=2)
                          .unsqueeze(1).broadcast_to([P, HL, DH // 2, 2]))
                    p1 = (pet[:, 1, :].rearrange("p (d t) -> p d t", t=2)
                          .unsqueeze(1).broadcast_to([P, HL, DH // 2, 2]))
                    t1 = pool_w512.tile([P, 512], BF16, tag="w512b")
                    t2 = pool_w512.tile([P, 512], BF16, tag="w512b")
                    t1v = t1.rearrange("p (h d t) -> p h d t", d=DH // 2, t=2)
                    t2v = t2.rearrange("p (h d t) -> p h d t", d=DH // 2, t=2)
                    nc.gpsimd.tensor_tensor(out=t1v, in0=qe, in1=p0, op=ALU.mult)
                    nc.gpsimd.tensor_tensor(out=t2v, in0=qo, in1=p1, op=ALU.mult)
                    nc.gpsimd.tensor_tensor(out=t1, in0=t1, in1=t2, op=ALU.add)
                    rq = pool_w512.tile([P, 512], BF16, tag="w512b")
                    rqv = rq.rearrange("p (h d) -> p h d", d=DH)
                    rsb = rs.unsqueeze(-1).broadcast_to([P, HL, DH])
                    nc.vector.tensor_tensor(
                        out=rqv, in0=t1.rearrange("p (h d) -> p h d", d=DH),
                        in1=rsb, op=ALU.mult)
                    # transpose head pairs -> qT/kT [128, pair, tok]
                    dst = qT if part == 0 else kT
                    pst = pool_psb.tile([P, 512], BF16, tag="ps512b")
                    for pr in range(NP):
                        nc.tensor.transpose(
                            pst[:, pr * P:(pr + 1) * P],
                            rq[:, pr * P:(pr + 1) * P], ident)
                    nc.vector.tensor_copy(
                        dst[:, :, ts], pst.rearrange("p (j t) -> p j t", j=NP))
            es_w1.close()

            # ---- Phase B1: linear1 mlp + gelu -> SBUF ----
            for ft in range(FT):
                for tn in range(4):
                    off = tn * 512
                    ps = pool_ps512.tile([P, 512], F32, tag="ps512")
                    for kd in range(KD):
                        nc.tensor.matmul(
                            ps, w1m[:, kd, ft * P:(ft + 1) * P],
                            xT[:, kd, off:off + 512],
                            start=(kd == 0), stop=(kd == KD - 1))
                    nc.scalar.activation(
                        mlp_sb[:, ft, off:off + 512], ps,
                        AF.Gelu_apprx_tanh)
            es_xT.close()

            # ---- Phase C: attention ----
            es_attn = ExitStack()
            pool_attn = es_attn.enter_context(
                tc.tile_pool(name="pattn", bufs=1))
            attnT = pool_attn.tile([P, NP, L], BF16, tag="attnT")
            w2sb = pool_attn.tile([P, KT2, DIM], BF16, tag="w2")
            nc.sync.dma_start(w2sb, w2.rearrange("(kt p) o -> p kt o", p=P))
            es_c = ExitStack()
            pool_ex = es_c.enter_context(tc.tile_pool(name="pex", bufs=6))
            pool_pacc = es_c.enter_context(
                tc.tile_pool(name="ppacc", bufs=2, space="PSUM"))
            for ph in range(NP):
                hA, hB = 2 * ph, 2 * ph + 1
                for qc in range(4):
                    qs_ = slice(qc * 512, (qc + 1) * 512)
                    pa = [pool_pacc.tile([P, 512], F32, tag="pacc",
                                         name=f"pa{i}") for i in range(2)]
                    for kt in range(16):
                        kts = slice(kt * P, (kt + 1) * P)
                        ex = []
                        for i, bp in enumerate((0, DH)):
                            pss = pool_ps512.tile([P, 512], F32, tag="ps512")
                            nc.tensor.matmul(
                                pss,
                                kT[bp:bp + DH, ph, kts],
                                qT[bp:bp + DH, ph, qs_],
                                start=True, stop=True)
                            e = pool_ex.tile([P, 512], BF16, tag="ex")
                            nc.scalar.activation(e, pss, AF.Exp, scale=0.125)
                            ex.append(e)
                        for i, h in enumerate((hA, hB)):
                            nc.tensor.matmul(
                                pa[i][0:DH + 1, :], v_sb[:, kt, h, :], ex[i],
                                start=(kt == 0), stop=(kt == 15))
                    for i, bp in enumerate((0, DH)):
                        rec = pool_w512.tile([P, 512], F32, tag="w512f")
                        nc.vector.reciprocal(
                            rec[DH:DH + 1, :], pa[i][DH:DH + 1, :])
                        recb = pool_w512.tile([P, 512], BF16, tag="w512b")
                        nc.vector.tensor_copy(
                            recb[DH:DH + 1, :], rec[DH:DH + 1, :])
                        bc = pool_ps512.tile([P, 512], F32, tag="ps512")
                        nc.tensor.matmul(
                            bc[bp:bp + DH, :], ones_sb[DH:DH + 1, 0:DH],
                            recb[DH:DH + 1, :], start=True, stop=True)
                        nc.scalar.copy(
                            attnT[bp:bp + DH, ph, qs_], pa[i][0:DH, :])
                        nc.vector.tensor_tensor(
                            out=attnT[bp:bp + DH, ph, qs_],
                            in0=attnT[bp:bp + DH, ph, qs_],
                            in1=bc[bp:bp + DH, :], op=ALU.mult)
            es_c.close()
            es_qkT.close()

            # ---- Phase E: linear2 (partial; host adds residual + b2) ----
            es_e = ExitStack()
            pool_y = es_e.enter_context(tc.tile_pool(name="py", bufs=3))
            for tt in range(TT):
                ts = slice(tt * P, (tt + 1) * P)
                yt = pool_y.tile([P, DIM], F32, tag="yt")
                for oc in range(2):
                    ps = pool_ps512.tile([P, 512], F32, tag="ps512")
                    for pr in range(AKT):
                        nc.tensor.matmul(
                            ps, attnT[:, pr, ts],
                            w2sb[:, pr, oc * 512:(oc + 1) * 512],
                            start=(pr == 0), stop=False)
                    for ft in range(FT):
                        nc.tensor.matmul(
                            ps, mlp_sb[:, ft, ts],
                            w2sb[:, AKT + ft, oc * 512:(oc + 1) * 512],
                            start=False, stop=(ft == FT - 1))
                    nc.vector.tensor_copy(yt[:, oc * 512:(oc + 1) * 512], ps)
                nc.sync.dma_start(y[ts, :], yt)
            es_e.close()
            es_attn.close()
            es_res.close()

        if repeat == 1:
            body()
        else:
            with tc.For_i(0, repeat, 1):
                body()

    nc.finalize()
    return nc


# ---------------- host side ----------------

_NC_CACHE = {}


def _get_nc(repeat=1):
    if repeat not in _NC_CACHE:
        _NC_CACHE[repeat] = gen_program(repeat)
    return _NC_CACHE[repeat]


def make_in_maps(x, pe, W1, b1, W2, b2, q_scale, k_scale):
    bf16 = mybir.dt.np(BF16)
    x = np.asarray(x, dtype=np.float32)
    pe = np.asarray(pe, dtype=np.float32)
    W1 = np.asarray(W1, dtype=np.float32)
    W2 = np.asarray(W2, dtype=np.float32)
    q_scale = np.asarray(q_scale, dtype=np.float32)
    k_scale = np.asarray(k_scale, dtype=np.float32)
    assert not np.any(np.asarray(b1)), "kernel assumes b1 == 0"

    pe_r = pe.reshape(L, DH // 2, 2, 2)

    def fold_pe(scale):
        s0 = np.repeat(scale[0::2], 2)  # scale for even input element
        s1 = np.repeat(scale[1::2], 2)
        p0 = pe_r[..., 0].reshape(L, DH) * s0[None, :]
        p1 = pe_r[..., 1].reshape(L, DH) * s1[None, :]
        return np.ascontiguousarray(
            np.stack([p0, p1], axis=1).astype(bf16))

    peq = fold_pe(q_scale)
    pek = fold_pe(k_scale)

    in_maps = []
    for c in range(8):
        b_idx, tp = c // 2, c % 2
        hs = tp * 512
        w1qkv = np.ascontiguousarray(np.concatenate(
            [W1[:, hs:hs + 512],
             W1[:, DIM + hs:DIM + hs + 512],
             W1[:, 2 * DIM + hs:2 * DIM + hs + 512]], axis=1).astype(bf16))
        w1mlp = np.ascontiguousarray(
            W1[:, 3 * DIM + tp * MLPL:3 * DIM + (tp + 1) * MLPL].astype(bf16))
        w2sh = np.ascontiguousarray(np.concatenate(
            [W2[hs:hs + 512, :],
             W2[DIM + tp * MLPL:DIM + (tp + 1) * MLPL, :]], axis=0).astype(bf16))
        in_maps.append({
            "xb": np.ascontiguousarray(x[b_idx]).astype(bf16),
            "peq": peq, "pek": pek,
            "w1qkv": w1qkv, "w1mlp": w1mlp, "w2": w2sh,
            "ident": np.eye(P, dtype=bf16),
        })
    return in_maps


def combine_outputs(results, x, b2):
    x = np.asarray(x, dtype=np.float32)
    b2 = np.asarray(b2, dtype=np.float32)
    y = np.empty((B, L, DIM), dtype=np.float32)
    for b_idx in range(B):
        y[b_idx] = (results[2 * b_idx]["y"] + results[2 * b_idx + 1]["y"]
                    + x[b_idx] + b2[None, :])
    return y


def kernel(x, pe, W1, b1, W2, b2, q_scale, k_scale):
    from concourse.bass_utils import run_bass_kernel_spmd
    nc = _get_nc(repeat=1)
    in_maps = make_in_maps(x, pe, W1, b1, W2, b2, q_scale, k_scale)
    res = run_bass_kernel_spmd(nc, in_maps, core_ids=list(range(8)))
    return combine_outputs(res.results, x, b2)


# revision 18
# speedup vs baseline: 4.4102x; 1.0285x over previous
"""Fused ParallelTransformerBlock kernel for 8 Trainium2 NeuronCores.

Sharding: Megatron-style tensor-parallel (2-way over heads + mlp_hidden)
x data-parallel (4-way over batch). Core c handles batch c//2 with
head/mlp shard c%2. Each core computes a partial output of linear2
(no residual, no bias); the host sums the two partials per batch and
adds x + b2 in fp32.

All tensors are bf16 except PSUM accumulation, layer/rms-norm stats and
the final output (fp32). All intermediates stay in SBUF (no DRAM
round-trips). QK^T matmuls for a head pair run row-tiled (base
partitions 0 and 64) so the two K=64 matmuls overlap on the PE array.
"""
import numpy as np

import concourse.bass as bass
import concourse.tile as tile
from concourse import bacc, mybir

DIM = 1024
L = 2048
B = 4
H = 16
DH = 64
MLP = 3072
EPS_LN = 1e-6
EPS_RMS = 1e-6

P = 128
KD = DIM // P          # 8 k-tiles over model dim
TT = L // P            # 16 token tiles
HL = H // 2            # 8 heads per core
NP = HL // 2           # 4 head pairs
MLPL = MLP // 2        # 1536 mlp columns per core
FT = MLPL // P         # 12 mlp feature tiles
AKT = HL * DH // P     # 4 attn k-tiles into linear2
KT2 = AKT + FT         # 16 linear2 k-tiles

F32 = mybir.dt.float32
BF16 = mybir.dt.bfloat16
AF = mybir.ActivationFunctionType
ALU = mybir.AluOpType
AX = mybir.AxisListType


def gen_program(repeat: int = 1):
    nc = bacc.Bacc("TRN2", target_bir_lowering=False, debug=False, num_devices=8)

    xb = nc.dram_tensor("xb", (L, DIM), BF16, kind="ExternalInput")
    peq = nc.dram_tensor("peq", (L, 2, DH), BF16, kind="ExternalInput")
    pek = nc.dram_tensor("pek", (L, 2, DH), BF16, kind="ExternalInput")
    w1qkv = nc.dram_tensor("w1qkv", (DIM, 3 * HL * DH), BF16, kind="ExternalInput")
    w1mlp = nc.dram_tensor("w1mlp", (DIM, MLPL), BF16, kind="ExternalInput")
    w2 = nc.dram_tensor("w2", (HL * DH + MLPL, DIM), BF16, kind="ExternalInput")
    ident_in = nc.dram_tensor("ident", (P, P), BF16, kind="ExternalInput")
    y = nc.dram_tensor("y", (L, DIM), F32, kind="ExternalOutput")

    from contextlib import ExitStack
    with tile.TileContext(nc) as tc, ExitStack() as es:
        pool_const = es.enter_context(tc.tile_pool(name="const", bufs=1))
        pool_w1024 = es.enter_context(tc.tile_pool(name="w1024", bufs=3))
        pool_w512 = es.enter_context(tc.tile_pool(name="w512", bufs=5))
        pool_small = es.enter_context(tc.tile_pool(name="small", bufs=8))
        pool_ps1024 = es.enter_context(tc.tile_pool(name="ps1024", bufs=2, space="PSUM"))
        pool_psb = es.enter_context(tc.tile_pool(name="psb", bufs=2, space="PSUM"))
        pool_pacc = es.enter_context(tc.tile_pool(name="ppacc", bufs=2, space="PSUM"))

        ident = pool_const.tile([P, P], BF16, tag="ident")
        nc.sync.dma_start(ident, ident_in[:, :])
        epsc = pool_const.tile([P, 1], F32, tag="epsc")
        nc.vector.memset(epsc, EPS_LN)
        ones_sb = pool_const.tile([P, DH], BF16, tag="ones_sb")
        nc.vector.memset(ones_sb, 1.0)

        def body():
            es_res = ExitStack()
            pool_res = es_res.enter_context(
                tc.tile_pool(name="pres", bufs=1, side="right"))
            # v laid out per (kt, head) as [tok-in-tile, kt, h, DH+1] with a
            # ones column at the end (accumulates the softmax denominator).
            v_sb = pool_res.tile([P, TT, HL, DH + 1], BF16, tag="v_sb")
            nc.vector.memset(v_sb[:, :, :, DH:DH + 1], 1.0)
            mlp_sb = pool_res.tile([P, FT, L], BF16, tag="mlp_sb")

            es_xT = ExitStack()
            pool_xT = es_xT.enter_context(tc.tile_pool(name="pxT", bufs=1))
            xT = pool_xT.tile([P, KD, L], BF16, tag="xT")
            w1m = pool_xT.tile([P, KD, MLPL], BF16, tag="w1m")
            nc.sync.dma_start(w1m, w1mlp.rearrange("(kt p) f -> p kt f", p=P))

            es_w1 = ExitStack()
            pool_wq = es_w1.enter_context(tc.tile_pool(name="pwq", bufs=1))
            w1sb = pool_wq.tile([P, KD, 3 * HL * DH], BF16, tag="w1sb")
            nc.sync.dma_start(w1sb, w1qkv.rearrange("(kt p) f -> p kt f", p=P))

            # ---- Phase A: LayerNorm + transpose to [dim, tok] ----
            for tt in range(TT):
                ts = slice(tt * P, (tt + 1) * P)
                xt = pool_w1024.tile([P, DIM], BF16, tag="w1024b")
                nc.sync.dma_start(xt, xb[ts, :])
                st = pool_small.tile([P, 2, 6], F32, tag="st")
                nc.vector.bn_stats(st[:, 0, :], xt[:, 0:512])
                nc.vector.bn_stats(st[:, 1, :], xt[:, 512:1024])
                mv = pool_small.tile([P, 2], F32, tag="mv")
                nc.vector.bn_aggr(mv, st)
                std = pool_small.tile([P, 1], F32, tag="std")
                nc.scalar.activation(std, mv[:, 1:2], AF.Sqrt, bias=epsc)
                rstd = pool_small.tile([P, 1], F32, tag="rstd")
                nc.vector.reciprocal(rstd, std)
                xln = pool_w1024.tile([P, DIM], BF16, tag="w1024b")
                nc.vector.tensor_scalar(
                    out=xln, in0=xt, scalar1=mv[:, 0:1], scalar2=rstd,
                    op0=ALU.subtract, op1=ALU.mult,
                )
                for g in range(2):
                    pst = pool_psb.tile([P, 512], BF16, tag="ps512b")
                    for j in range(4):
                        kd = g * 4 + j
                        nc.tensor.transpose(
                            pst[:, j * P:(j + 1) * P],
                            xln[:, kd * P:(kd + 1) * P], ident)
                    nc.vector.tensor_copy(
                        xT[:, g * 4:(g + 1) * 4, ts],
                        pst.rearrange("p (j t) -> p j t", j=4))

            # ---- Phase B2: linear1 qkv + rmsnorm + rope + transpose ----
            es_qkT = ExitStack()
            pool_qkT = es_qkT.enter_context(
                tc.tile_pool(name="pqkT", bufs=1, side="right"))
            qT = pool_qkT.tile([P, NP, L], BF16, tag="qT")
            kT = pool_qkT.tile([P, NP, L], BF16, tag="kT")

            for part in range(3):  # 0=q, 1=k, 2=v
                for tt in range(TT):
                    ts = slice(tt * P, (tt + 1) * P)
                    psw = pool_ps1024.tile([P, 1024], F32, tag="ps1024", name="psw")
                    ps = psw[:, 0:512]
                    for kd in range(KD):
                        nc.tensor.matmul(
                            ps, xT[:, kd, ts],
                            w1sb[:, kd, part * 512:(part + 1) * 512],
                            start=(kd == 0), stop=(kd == KD - 1))
                    if part == 2:  # v
                        nc.scalar.copy(
                            v_sb[:, tt, :, 0:DH],
                            ps.rearrange("p (h d) -> p h d", d=DH))
                        continue
                    # rmsnorm stats
                    sq = pool_w512.tile([P, 512], F32, tag="w512f")
                    nc.vector.tensor_tensor(out=sq, in0=ps, in1=ps, op=ALU.mult)
                    ss = pool_small.tile([P, HL], F32, tag="ss")
                    nc.vector.tensor_reduce(
                        ss, sq.rearrange("p (h d) -> p h d", d=DH),
                        axis=AX.X, op=ALU.add)
                    sd = pool_small.tile([P, HL], F32, tag="sd")
                    nc.scalar.activation(sd, ss, AF.Sqrt, scale=1.0 / DH,
                                         bias=epsc)
                    rs = pool_small.tile([P, HL], F32, tag="rs")
                    nc.vector.reciprocal(rs, sd)
                    # rope (pe pre-folded with q/k head scales on host)
                    pe_src = peq if part == 0 else pek
                    pet = pool_small.tile([P, 2, DH], BF16, tag="pet")
                    nc.sync.dma_start(pet, pe_src[ts, :, :])
                    qs = pool_w512.tile([P, 512], BF16, tag="w512b")
                    nc.vector.tensor_copy(qs, ps)
                    qs4 = qs.rearrange("p (h d t) -> p h d t", d=DH // 2, t=2)
                    qe = qs4[:, :, :, 0].unsqueeze(-1).broadcast_to([P, HL, DH // 2, 2])
                    qo = qs4[:, :, :, 1].unsqueeze(-1).broadcast_to([P, HL, DH // 2, 2])
                    p0 = (pet[:, 0, :].rearrange("p (d t) -> p d t", t=2)
                          .unsqueeze(1).broadcast_to([P, HL, DH // 2, 2]))
                    p1 = (pet[:, 1, :].rearrange("p (d t) -> p d t", t=2)
                          .unsqueeze(1).broadcast_to([P, HL, DH // 2, 2]))
                    t1 = pool_w512.tile([P, 512], BF16, tag="w512b")
                    t2 = pool_w512.tile([P, 512], BF16, tag="w512b")
                    t1v = t1.rearrange("p (h d t) -> p h d t", d=DH // 2, t=2)
                    t2v = t2.rearrange("p (h d t) -> p h d t", d=DH // 2, t=2)
                    nc.gpsimd.tensor_tensor(out=t1v, in0=qe, in1=p0, op=ALU.mult)
                    nc.gpsimd.tensor_tensor(out=t2v, in0=qo, in1=p1, op=ALU.mult)
                    nc.gpsimd.tensor_tensor(out=t1, in0=t1, in1=t2, op=ALU.add)
                    rq = pool_w512.tile([P, 512], BF16, tag="w512b")
                    rqv = rq.rearrange("p (h d) -> p h d", d=DH)
                    rsb = rs.unsqueeze(-1).broadcast_to([P, HL, DH])
                    nc.vector.tensor_tensor(
                        out=rqv, in0=t1.rearrange("p (h d) -> p h d", d=DH),
                        in1=rsb, op=ALU.mult)
                    # transpose head pairs -> qT/kT [128, pair, tok]
                    dst = qT if part == 0 else kT
                    pst = pool_psb.tile([P, 512], BF16, tag="ps512b")
                    for pr in range(NP):
                        nc.tensor.transpose(
                            pst[:, pr * P:(pr + 1) * P],
                            rq[:, pr * P:(pr + 1) * P], ident)
                    nc.vector.tensor_copy(
                        dst[:, :, ts], pst.rearrange("p (j t) -> p j t", j=NP))
            es_w1.close()

            # ---- Phase B1: linear1 mlp + gelu -> SBUF ----
            for ft in range(FT):
                for tm in range(2):
                    ps = pool_ps1024.tile([P, 1024], F32, tag="ps1024")
                    for tn in range(2):
                        off = tm * 1024 + tn * 512
                        for kd in range(KD):
                            nc.tensor.matmul(
                                ps[:, tn * 512:(tn + 1) * 512],
                                w1m[:, kd, ft * P:(ft + 1) * P],
                                xT[:, kd, off:off + 512],
                                start=(kd == 0), stop=(kd == KD - 1))
                    nc.scalar.activation(
                        mlp_sb[:, ft, tm * 1024:(tm + 1) * 1024], ps,
                        AF.Gelu_apprx_tanh)
            es_xT.close()

            # ---- Phase C: attention ----
            es_attn = ExitStack()
            pool_attn = es_attn.enter_context(
                tc.tile_pool(name="pattn", bufs=1))
            attnT = pool_attn.tile([P, NP, L], BF16, tag="attnT")
            w2sb = pool_attn.tile([P, KT2, DIM], BF16, tag="w2")
            nc.sync.dma_start(w2sb, w2.rearrange("(kt p) o -> p kt o", p=P))
            es_c = ExitStack()
            pool_ex = es_c.enter_context(tc.tile_pool(name="pex", bufs=4))
            for ph in range(NP):
                hA, hB = 2 * ph, 2 * ph + 1
                for qc in range(4):
                    qs_ = slice(qc * 512, (qc + 1) * 512)
                    pa = [pool_pacc.tile([P, 512], F32, tag="pacc",
                                         name=f"pa{i}") for i in range(2)]
                    for kt in range(16):
                        kts = slice(kt * P, (kt + 1) * P)
                        pss = pool_ps1024.tile([P, 1024], F32, tag="ps1024")
                        for i, bp in enumerate((0, DH)):
                            nc.tensor.matmul(
                                pss[:, i * 512:(i + 1) * 512],
                                kT[bp:bp + DH, ph, kts],
                                qT[bp:bp + DH, ph, qs_],
                                start=True, stop=True)
                        ex = pool_ex.tile([P, 1024], BF16, tag="ex")
                        nc.scalar.activation(ex, pss, AF.Exp, scale=0.125)
                        for i, h in enumerate((hA, hB)):
                            nc.tensor.matmul(
                                pa[i][0:DH + 1, :], v_sb[:, kt, h, :],
                                ex[:, i * 512:(i + 1) * 512],
                                start=(kt == 0), stop=(kt == 15))
                    bc = pool_ps1024.tile([P, 1024], F32, tag="ps1024")
                    for i, bp in enumerate((0, DH)):
                        rec = pool_w512.tile([P, 512], F32, tag="w512f")
                        nc.vector.reciprocal(
                            rec[DH:DH + 1, :], pa[i][DH:DH + 1, :])
                        recb = pool_w512.tile([P, 512], BF16, tag="w512b")
                        nc.vector.tensor_copy(
                            recb[DH:DH + 1, :], rec[DH:DH + 1, :])
                        nc.tensor.matmul(
                            bc[bp:bp + DH, i * 512:(i + 1) * 512],
                            ones_sb[DH:DH + 1, 0:DH],
                            recb[DH:DH + 1, :], start=True, stop=True)
                        nc.scalar.copy(
                            attnT[bp:bp + DH, ph, qs_], pa[i][0:DH, :])
                        nc.vector.tensor_tensor(
                            out=attnT[bp:bp + DH, ph, qs_],
                            in0=attnT[bp:bp + DH, ph, qs_],
                            in1=bc[bp:bp + DH, i * 512:(i + 1) * 512],
                            op=ALU.mult)
            es_c.close()
            es_qkT.close()

            # ---- Phase E: linear2 (partial; host adds residual + b2) ----
            es_e = ExitStack()
            pool_y = es_e.enter_context(tc.tile_pool(name="py", bufs=3))
            for tt in range(TT):
                ts = slice(tt * P, (tt + 1) * P)
                yt = pool_y.tile([P, DIM], F32, tag="yt")
                ps = pool_ps1024.tile([P, 1024], F32, tag="ps1024")
                for oc in range(2):
                    pso = ps[:, oc * 512:(oc + 1) * 512]
                    for pr in range(AKT):
                        nc.tensor.matmul(
                            pso, attnT[:, pr, ts],
                            w2sb[:, pr, oc * 512:(oc + 1) * 512],
                            start=(pr == 0), stop=False)
                    for ft in range(FT):
                        nc.tensor.matmul(
                            pso, mlp_sb[:, ft, ts],
                            w2sb[:, AKT + ft, oc * 512:(oc + 1) * 512],
                            start=False, stop=(ft == FT - 1))
                nc.vector.tensor_copy(yt, ps)
                nc.sync.dma_start(y[ts, :], yt)
            es_e.close()
            es_attn.close()
            es_res.close()

        if repeat == 1:
            body()
        else:
            with tc.For_i(0, repeat, 1):
                body()

    nc.finalize()
    return nc


# ---------------- host side ----------------

_NC_CACHE = {}


def _get_nc(repeat=1):
    if repeat not in _NC_CACHE:
        _NC_CACHE[repeat] = gen_program(repeat)
    return _NC_CACHE[repeat]


def make_in_maps(x, pe, W1, b1, W2, b2, q_scale, k_scale):
    bf16 = mybir.dt.np(BF16)
    x = np.asarray(x, dtype=np.float32)
    pe = np.asarray(pe, dtype=np.float32)
    W1 = np.asarray(W1, dtype=np.float32)
    W2 = np.asarray(W2, dtype=np.float32)
    q_scale = np.asarray(q_scale, dtype=np.float32)
    k_scale = np.asarray(k_scale, dtype=np.float32)
    assert not np.any(np.asarray(b1)), "kernel assumes b1 == 0"

    pe_r = pe.reshape(L, DH // 2, 2, 2)

    def fold_pe(scale):
        s0 = np.repeat(scale[0::2], 2)  # scale for even input element
        s1 = np.repeat(scale[1::2], 2)
        p0 = pe_r[..., 0].reshape(L, DH) * s0[None, :]
        p1 = pe_r[..., 1].reshape(L, DH) * s1[None, :]
        return np.ascontiguousarray(
            np.stack([p0, p1], axis=1).astype(bf16))

    peq = fold_pe(q_scale)
    pek = fold_pe(k_scale)

    in_maps = []
    for c in range(8):
        b_idx, tp = c // 2, c % 2
        hs = tp * 512
        w1qkv = np.ascontiguousarray(np.concatenate(
            [W1[:, hs:hs + 512],
             W1[:, DIM + hs:DIM + hs + 512],
             W1[:, 2 * DIM + hs:2 * DIM + hs + 512]], axis=1).astype(bf16))
        w1mlp = np.ascontiguousarray(
            W1[:, 3 * DIM + tp * MLPL:3 * DIM + (tp + 1) * MLPL].astype(bf16))
        w2sh = np.ascontiguousarray(np.concatenate(
            [W2[hs:hs + 512, :],
             W2[DIM + tp * MLPL:DIM + (tp + 1) * MLPL, :]], axis=0).astype(bf16))
        in_maps.append({
            "xb": np.ascontiguousarray(x[b_idx]).astype(bf16),
            "peq": peq, "pek": pek,
            "w1qkv": w1qkv, "w1mlp": w1mlp, "w2": w2sh,
            "ident": np.eye(P, dtype=bf16),
        })
    return in_maps


def combine_outputs(results, x, b2):
    x = np.asarray(x, dtype=np.float32)
    b2 = np.asarray(b2, dtype=np.float32)
    y = np.empty((B, L, DIM), dtype=np.float32)
    for b_idx in range(B):
        y[b_idx] = (results[2 * b_idx]["y"] + results[2 * b_idx + 1]["y"]
                    + x[b_idx] + b2[None, :])
    return y


def kernel(x, pe, W1, b1, W2, b2, q_scale, k_scale):
    from concourse.bass_utils import run_bass_kernel_spmd
    nc = _get_nc(repeat=1)
    in_maps = make_in_maps(x, pe, W1, b1, W2, b2, q_scale, k_scale)
    res = run_bass_kernel_spmd(nc, in_maps, core_ids=list(range(8)))
    return combine_outputs(res.results, x, b2)
